# revision 22
# baseline (speedup 1.0000x reference)
"""Trainium2 Bass kernel for nn_KVCacheMemory (dual-attention memory gate).

Data-parallel over batch: each of the 8 NeuronCores computes one batch's two
single-head SxS attentions (S=4096, D=192) plus the flag-gated combine.

Weight-folded formulation (host folds the tiny weight pairs, device keeps all
x-dependent FLOPs):
  scores_a = x @ M_a @ x^T        with M_a = Wq_a^T Wk_a  (scaled by SC for fp8)
  result_a = softmax(scores_a) @ x @ C_a   with C_a = Wv_a^T Wo_a^T
so the k/v projections disappear: x itself (fp8, two layouts) is both the key
stationary operand of the scores matmul and the value stationary operand of
the AV matmul, shared by the two attentions. Per-core dataflow (all
contractions on the TensorEngine; no on-device transposes):
  - scoresT[k,q] = xT.T @ qM computed in the transposed layout so the exp()
    output (ACT, 1/(SC*sqrt(D)) folded into the activation scale) is already
    the moving operand of the oT accumulation matmul.
  - A ones-column appended to x (host-side) makes the softmax row-sum fall
    out of the oT matmul as an extra row; a unit column appended to C carries
    it through the output projection, so it lands as column 192 of the final
    [128,193] PSUM tile, per-partition aligned for one reciprocal + fused
    scalar multiply (softmax normalization commutes with the linear C).
"""
import numpy as np
import ml_dtypes

import concourse.bacc as bacc
import concourse.tile as tile
import concourse.mybir as mybir
from concourse.bass_utils import run_bass_kernel_spmd

B, S, D = 8, 4096, 192
MEM_READ, MEM_WRITE, MEM_READY = 156, 157, 158
P = 128          # partitions / tile rows
QB = 512         # q block (matmul free dim / PSUM bank)
NQB = S // QB    # 8
KC = 128         # key chunk (contraction tile)
NKC = S // KC    # 32
NT = S // P      # 32 row tiles
D0, D1 = 128, 64  # output split of D=192 for the AV matmul
SC = 64.0        # fp8 range scale folded into M
SCALE = 1.0 / (SC * float(np.sqrt(D)))
F32 = mybir.dt.float32
BF16 = mybir.dt.bfloat16
FP8 = mybir.dt.float8e4
DR = mybir.MatmulPerfMode.DoubleRow
VBLK = 208       # x_ext block stride (16B-aligned for DoubleRow lhsT step)
N_CORES = 8

_CACHE = {}


def _build():
    nc = bacc.Bacc("TRN2", target_bir_lowering=False, debug=False,
                   num_devices=N_CORES)
    x = nc.dram_tensor("x", [S, D], F32, kind="ExternalInput").ap()
    xt8 = nc.dram_tensor("xt8", [96, 2 * S], FP8, kind="ExternalInput").ap()
    xe8 = nc.dram_tensor("xe8", [P, NT * VBLK], FP8, kind="ExternalInput").ap()
    m8 = nc.dram_tensor("m8", [96, 2 * 2 * D], FP8, kind="ExternalInput").ap()
    coedr = nc.dram_tensor("coedr", [97, 2 * 2 * VBLK], FP8, kind="ExternalInput").ap()
    esc = nc.dram_tensor("esc", [97, 2], F32, kind="ExternalInput").ap()
    params = nc.dram_tensor("params", [P, 4], F32, kind="ExternalInput").ap()
    out = nc.dram_tensor("out", [S, D], F32, kind="ExternalOutput").ap()

    with tile.TileContext(nc) as tc:
        _emit(nc, tc, x, xt8, xe8, m8, coedr, esc, params, out)
    nc.compile()
    return nc


def _emit(nc, tc, x, xt8, xe8, m8, coedr, esc, params, out):
    from contextlib import ExitStack
    with ExitStack() as st:
        cpool = st.enter_context(tc.tile_pool(name="const", bufs=1))
        bigpool = st.enter_context(tc.tile_pool(name="big", bufs=1))
        apool = st.enter_context(tc.tile_pool(name="attn", bufs=6))
        opool = st.enter_context(tc.tile_pool(name="osb", bufs=2))
        xpool = st.enter_context(tc.tile_pool(name="xin", bufs=3))
        tpool = st.enter_context(tc.tile_pool(name="tmp", bufs=3))
        # PSUM budget (8 banks): mm 3x[128,1024]=6, oT0+oT1 1x each=2;
        # res tiles rotate through the oT0 slot (tag-shared, freed post-copy)
        mmpool = st.enter_context(tc.tile_pool(name="mm", bufs=3, space="PSUM"))
        oaccpool = st.enter_context(tc.tile_pool(name="oacc", bufs=1, space="PSUM"))

        # resident constants / activations. Small gating tensors first; xt8
        # loads chunked per (half, sb) so qproj unit sb only waits for its
        # own slices.
        m8s = cpool.tile([96, 2 * 2 * D], FP8, tag="m8s")
        nc.sync.dma_start(m8s, m8)
        # xt8 blocked layout [96, sb, o, 512]: one 2-sb DMA covers the two
        # DR halves of those q-blocks contiguously.
        xt8s = cpool.tile([96, 2 * S], FP8, tag="xt8s")
        for sb2 in range(4):
            sl = slice(sb2 * 2 * 2 * QB, (sb2 + 1) * 2 * 2 * QB)
            nc.sync.dma_start(xt8s[:, sl], xt8[:, sl])
        xe8s = cpool.tile([P, NT * VBLK], FP8, tag="xe8s")
        for c in range(2):
            sl = slice(c * 16 * VBLK, (c + 1) * 16 * VBLK)
            nc.sync.dma_start(xe8s[:, sl], xe8[:, sl])
        coedrs = cpool.tile([97, 2 * 2 * VBLK], FP8, tag="coedrs")
        nc.sync.dma_start(coedrs, coedr)
        escs = cpool.tile([97, 2], F32, tag="escs")
        nc.sync.dma_start(escs, esc)
        pp = cpool.tile([P, 4], F32, tag="pp")
        nc.sync.dma_start(pp, params)
        # pre-fault the exp ACT table so the ~2.7us load overlaps input DMAs
        warm = cpool.tile([1, 1], F32, tag="warm")
        nc.scalar.activation(warm, m8s[0:1, 0:1],
                             mybir.ActivationFunctionType.Exp)

        # out accumulator [128, 32*192] f32 (tile g lives at cols g*192)
        out_acc = bigpool.tile([P, NT * D], F32, tag="out_acc")
        # epilogue fp8 pack buffers (ping-pong); pad row zeroed once - the
        # per-epi casts only ever write rows 0..96 / 0..95
        oTsb = [bigpool.tile([97, 2 * QB], FP8, tag=f"oTs{i}", name="oTsb")
                for i in range(2)]
        nc.gpsimd.memset(oTsb[0][96:97, QB:2 * QB], 0.0)
        nc.gpsimd.memset(oTsb[1][96:97, QB:2 * QB], 0.0)

        # per-attention qM buffers, blocked [96, sb, o, 512] like xt8
        qMd = [bigpool.tile([96, 2 * S], FP8, tag=f"qMd{a}", name="qMd")
               for a in range(2)]
        m83 = m8s.rearrange("p (o c) -> p o c", o=2)
        coedr4 = coedrs.rearrange("p (a o v) -> p a o v", a=2, v=VBLK)
        xt84 = xt8s.rearrange("p (b o c) -> p b o c", o=2, c=QB)
        qM4 = [q.rearrange("p (b o c) -> p b o c", o=2, c=QB) for q in qMd]
        xe83 = xe8s.rearrange("p (t c) -> p t c", c=VBLK)

        def qproj_unit(att, sb, h):
            """qM[:, sb, h, :] via one fp8-DR matmul."""
            ps = mmpool.tile([P, QB], F32, tag="mm", name="ps_proj")
            nc.tensor.matmul(
                ps[:96, :],
                m83[:, :, att * D + h * 96:att * D + (h + 1) * 96],
                xt84[:, sb], start=True, stop=True, perf_mode=DR)
            nc.vector.tensor_copy(qM4[att][:, sb, h], ps[:96, :])

        NPR = NKC // 2
        ostate = {}
        pending_warm = {}

        def emit_pair(att, qb, pr):
            """Score matmuls + exp for key-chunk pair pr of (att, qb)."""
            qs3 = qM4[att][:, qb]
            sc = mmpool.tile([P, 2 * QB], F32, tag="mm", name="sc")
            for h in range(2):
                kc = 2 * pr + h
                nc.tensor.matmul(sc[:, h * QB:(h + 1) * QB],
                                 xt84[:, kc // 4, :,
                                      (kc % 4) * KC:(kc % 4 + 1) * KC],
                                 qs3, start=True, stop=True,
                                 perf_mode=DR)
            at = apool.tile([P, 2 * QB], FP8, tag="at")
            nc.scalar.activation(at, sc, mybir.ActivationFunctionType.Exp,
                                 scale=SCALE)
            return at

        def emit_warm(att, qb):
            # first pair of the NEXT q-block, emitted under the current one
            # so the exp pipeline never drains at the boundary
            pending_warm[(att, qb)] = emit_pair(att, qb, 0)

        def phaseB_main(att, qb, interleave=None):
            oT0 = oaccpool.tile([97, QB], F32, tag="oT0x")
            oT1 = oaccpool.tile([96, QB], F32, tag="oT1x")
            ostate[(att, qb)] = (oT0, oT1)
            for pr in range(NPR):
                if pr == 0 and (att, qb) in pending_warm:
                    at = pending_warm.pop((att, qb))
                else:
                    at = emit_pair(att, qb, pr)
                at3 = at.rearrange("p (o n) -> p o n", o=2)
                nc.tensor.matmul(oT0, xe83[:, 2 * pr:2 * pr + 2, 0:97], at3,
                                 start=(pr == 0), stop=(pr == NPR - 1),
                                 perf_mode=DR)
                nc.tensor.matmul(oT1, xe83[:, 2 * pr:2 * pr + 2, 97:D + 1],
                                 at3, start=(pr == 0), stop=(pr == NPR - 1),
                                 perf_mode=DR)
                if interleave is not None:
                    interleave(pr)

        def epi_casts(att, qb):
            # pack both AV tiles into the two DR halves of one fp8 buffer
            # (scales 1/8 on dims, 1/1024 on the rowsum row; compensated in
            # coedr's x64 and the 1/128 folded into the gate params)
            oT0, oT1 = ostate.pop((att, qb))
            oTs = oTsb[(att * NQB + qb) % 2]
            with tc.high_priority(offset=200):
                nc.vector.tensor_scalar(oTs[:, 0:QB], oT0, escs[:, 0:1], None,
                                        op0=mybir.AluOpType.mult)
                nc.vector.tensor_scalar(oTs[0:96, QB:2 * QB], oT1,
                                        escs[0:96, 1:2], None,
                                        op0=mybir.AluOpType.mult)
            ostate[(att, qb, "oTs")] = oTs

        def epi_qt(att, qb, qt, last=False):
            flag_col = 1 + att
            oTs3 = ostate[(att, qb, "oTs")].rearrange("p (o n) -> p o n", o=2)
            g = qb * 4 + qt
            if last:
                res = mmpool.tile([P, QB], F32, tag="mm", name="res")
            else:
                res = oaccpool.tile([P, QB], F32, tag="oT0x", name="res")
            res = res[:, 0:D + 1]
            nc.tensor.matmul(res, oTs3[:, :, qt * P:(qt + 1) * P],
                             coedr4[:, att, :, 0:D + 1],
                             start=True, stop=True, perf_mode=DR)
            rec = tpool.tile([P, 1], F32, tag="rec")
            nc.vector.reciprocal(rec, res[:, D:D + 1])
            tmp = tpool.tile([P, D], F32, tag="tmp")
            nc.vector.tensor_scalar(
                tmp, res[:, 0:D], rec, pp[:, flag_col:flag_col + 1],
                op0=mybir.AluOpType.mult, op1=mybir.AluOpType.mult)
            acc = out_acc[:, g * D:(g + 1) * D]
            if att == 0:
                xt = xpool.tile([P, D], F32, tag="xt")
                nc.sync.dma_start(xt, x[g * P:(g + 1) * P, :])
                nc.vector.tensor_scalar(
                    acc, xt, pp[:, 0:1], None, op0=mybir.AluOpType.mult)
                nc.vector.tensor_add(acc, acc, tmp)
            else:
                nc.vector.tensor_add(acc, acc, tmp)
                nc.vector.memset(acc[:, MEM_READ:MEM_WRITE + 1], 0.0)
                nc.vector.tensor_copy(acc[:, MEM_READY:MEM_READY + 1],
                                      pp[:, 3:4])
                nc.sync.dma_start(out[g * P:(g + 1) * P, :], acc)
            if qt == 3:
                del ostate[(att, qb, "oTs")]

        def phaseB_epi(att, qb, last=False):
            epi_casts(att, qb)
            for qt in range(4):
                epi_qt(att, qb, qt, last=last)

        # driver: qproj(0, sb 0) upfront; remaining att0 units spread under
        # B(0,0), att1 units under B(0,1..2); epilogues deferred one qb so
        # the next qb's score matmuls keep ACT fed.
        qproj_unit(0, 0, 0)
        qproj_unit(0, 0, 1)

        def ilv0(pr):
            if 1 <= pr <= 14:
                qproj_unit(0, (pr + 1) // 2, (pr + 1) % 2)

        def ilv_att1(qb):
            def f(pr):
                if pr % 2 == 0:
                    u = (qb - 1) * 8 + pr // 2
                    qproj_unit(1, u // 2, u % 2)
            return f

        def with_warm(base, nxt):
            def f(pr):
                if base is not None:
                    base(pr)
                if nxt is not None and pr == 14:
                    emit_warm(*nxt)
            return f

        phaseB_main(0, 0, interleave=with_warm(ilv0, (0, 1)))
        for qb in range(1, NQB):
            ilv = ilv_att1(qb) if qb in (1, 2) else None
            nxt = (0, qb + 1) if qb < NQB - 1 else (1, 0)
            phaseB_main(0, qb, interleave=with_warm(ilv, nxt))
            phaseB_epi(0, qb - 1)
        for qb in range(NQB - 1):
            phaseB_main(1, qb, interleave=with_warm(None, (1, qb + 1)))
            phaseB_epi(0 if qb == 0 else 1, NQB - 1 if qb == 0 else qb - 1)
        # last q-block: fold the previous epilogue into its pair loop,
        # one query-tile per third pair so the res tiles never wedge the
        # mm ring; only the final epilogue remains post-loop
        def ilv_last(pr):
            if pr == 2:
                epi_casts(1, NQB - 2)
            if pr in (3, 6, 9, 12):
                epi_qt(1, NQB - 2, (pr - 3) // 3, last=True)
        phaseB_main(1, NQB - 1, interleave=ilv_last)
        phaseB_epi(1, NQB - 1, last=True)


def _prep_core_inputs(x_full, weights):
    """Host-side shard/layout prep. weights: dict of the 8 [192,192] f32."""
    f8 = ml_dtypes.float8_e4m3
    # folded weight matrices (weights-only algebra; all x-compute on device)
    m8 = np.zeros((96, 2, 2 * D), np.float32)
    coedr = np.zeros((97, 2, 2, VBLK), np.float32)
    for a, (qn, kn, vn, on) in enumerate(
            (("Wq_r", "Wk_r", "Wv_r", "Wo_r"),
             ("Wq_w", "Wk_w", "Wv_w", "Wo_w"))):
        M = SC * (weights[qn].T.astype(np.float64)
                  @ weights[kn].astype(np.float64))   # [d, e]
        for o in range(2):
            m8[:, o, a * D:(a + 1) * D] = M[o * 96:(o + 1) * 96, :]
        C = (weights[vn].T.astype(np.float64)
             @ weights[on].T.astype(np.float64))      # [d, o]
        # DR epilogue moving operand: row m = o*97+i covers [192 dims,
        # rowsum-carry, pad], scaled x64 to sit in fp8 range
        cx = np.zeros((194, D + 1), np.float32)
        cx[:D, :D] = SC * C
        cx[D, D] = SC
        for o in range(2):
            coedr[:, a, o, :D + 1] = cx[o * 97:o * 97 + 97, :]
    m8 = np.clip(m8, -240, 240).astype(f8).reshape(96, 2 * 2 * D)
    coedr = np.clip(coedr, -240, 240).astype(f8).reshape(97, 2 * 2 * VBLK)
    # epilogue cast scales: 1/8 on attention dims, 1/1024 on the rowsum row
    esc = np.full((97, 2), 1.0 / 8, np.float32)
    esc[95, 1] = 1.0 / 1024
    esc[96, 1] = 0.0
    in_maps = []
    for c in range(N_CORES):
        xb = np.ascontiguousarray(x_full[c]).astype(np.float32)  # [4096,192]
        x8 = np.clip(xb, -240, 240).astype(f8)
        # xt8 blocked: [96, sb, o, 512] with feature = 96*o + i
        xt8 = np.ascontiguousarray(
            x8.reshape(NQB, QB, 2, 96).transpose(3, 0, 2, 1)
        ).reshape(96, 2 * S)
        # xe8: [128, NT, VBLK]: keys tile t on partitions, dims + ones col
        xe8 = np.zeros((P, NT, VBLK), f8)
        xe8[:, :, :D] = x8.reshape(NT, P, D).transpose(1, 0, 2)
        xe8[:, :, D] = np.float32(1.0)
        rg = float(xb[0, MEM_READ])
        wg = float(xb[0, MEM_WRITE])
        # gate flags pre-divided by 128 = (1/8)*(16) epilogue scale product
        pvec = np.array([1.0 - rg - wg, rg / 128, wg / 128, rg + wg],
                        np.float32)
        in_maps.append({
            "x": xb,
            "xt8": xt8,
            "xe8": np.ascontiguousarray(xe8.reshape(P, NT * VBLK)),
            "m8": m8,
            "coedr": coedr,
            "esc": esc,
            "params": np.tile(pvec, (P, 1)),
        })
    return in_maps


def _run(inputs, **spmd_kwargs):
    if "nc" not in _CACHE:
        _CACHE["nc"] = _build()
    nc = _CACHE["nc"]
    x_full = np.asarray(inputs["x"], np.float32)
    weights = {k: np.asarray(inputs[k], np.float32) for k in
               ("Wq_r", "Wk_r", "Wv_r", "Wo_r", "Wq_w", "Wk_w", "Wv_w", "Wo_w")}
    in_maps = _prep_core_inputs(x_full, weights)
    res = run_bass_kernel_spmd(nc, in_maps, list(range(N_CORES)), **spmd_kwargs)
    out = np.stack([res.results[c]["out"] for c in range(N_CORES)], axis=0)
    return out.astype(np.float32), res


def kernel(**inputs):
    out, _ = _run(inputs)
    return out


def kernel_traced(**inputs):
    """For test.py: also returns BassKernelResults with profile info."""
    return _run(inputs, trace=True)
